# revision 54
# baseline (speedup 1.0000x reference)
"""DeepSeekV3-style MoE block on 8 Trainium2 NeuronCores.

Strategy (expert-parallel, host-routed dispatch/combine):
  - Host computes the (tiny) sigmoid gate in fp32 numpy, does top-2 selection
    and builds per-expert token lists (the "all-to-all dispatch" happens while
    sharding the inputs).
  - Core e runs expert e's SwiGLU over its gathered tokens (padded to the
    exact max expert load) plus a 1/8 token-slice of the shared expert, all
    in bf16 on the TensorEngine with fp32 PSUM accumulation.
  - The per-token gate weight is applied on the HOST during the combine
    scatter-add (it is a per-column scale of the core's output), so the
    device program is a pure unscaled SwiGLU.
  - The host scatter-adds the per-core outputs back into the full [B,S,H]
    tensor (the "combine" happens while unsharding).

Matmul layouts (no on-chip transposes):
  phase A:  act[f,c] = silu(w1[h,f].T @ x[h,c]) * (w3[h,f].T @ x[h,c])
            (w1/w3 matmuls interleaved per k-tile so each freshly-landed x
            piece feeds two back-to-back matmuls — matches the DMA issue
            rate at f=0 when x is still streaming in)
  phase B:  y[h,c]   = w2[f,h].T @ act[f,c]     (h-major output: cost tracks
            the exact token count instead of 128-rounded token tiles)
Host-side pre-tiling puts every DRAM operand in [128, ...] partition-major
layout so each DMA is contiguous.
"""

import hashlib
import os
import sys

for _p in ("/opt/trn_rl_repo", "/opt/pypackages"):
    if _p not in sys.path:
        sys.path.append(_p)

from contextlib import ExitStack

import numpy as np
import ml_dtypes

import concourse.bacc as bacc
import concourse.mybir as mybir
import concourse.tile as tile
from concourse import bass2jax
from concourse.bass_utils import run_bass_kernel_spmd

_NEFF_CACHE_DIR = os.path.expanduser("~/.cache/bass_neff_cache")
_active_build_key = None   # set by _get_nc around the PJRT dispatch


def _install_neff_cache():
    """Persist the compiled bass_exec NEFF across processes.

    The walrus backend takes minutes for this kernel and has no cache of its
    own.  The HLO bytes are not byte-stable across processes (volatile ids /
    debug metadata), so the cache key is derived from the *build inputs*
    (capacities + CFG + build source) instead.  Only the renamed NEFF bytes
    are stored; each request re-wraps them around its own HLO."""
    if getattr(bass2jax, "_ant_neff_cache_wrapped", False):
        return
    inner = bass2jax.neuronx_cc_hook

    captured = {}
    orig_rename = bass2jax.rename_neff_tensors_and_patch_header

    def capture_rename(neff_path, mapping):
        data = orig_rename(neff_path, mapping)
        captured["neff"] = data
        return data

    bass2jax.rename_neff_tensors_and_patch_header = capture_rename

    def cached_hook(code, code_format, platform_version, file_prefix):
        c = code if isinstance(code, (bytes, bytearray)) else str(code).encode()
        if b"bass_exec" not in c or _active_build_key is None:
            return inner(code, code_format, platform_version, file_prefix)
        from libneuronxla.libncc import _wrap_neff_as_custom_call

        path = os.path.join(_NEFF_CACHE_DIR, _active_build_key + ".neff")
        try:
            if os.path.exists(path):
                with open(path, "rb") as f:
                    return 0, _wrap_neff_as_custom_call(bytes(c), f.read())
        except Exception:
            pass
        captured.pop("neff", None)
        r = inner(code, code_format, platform_version, file_prefix)
        neff = captured.pop("neff", None)
        if neff is not None:
            try:
                os.makedirs(_NEFF_CACHE_DIR, exist_ok=True)
                tmp = f"{path}.tmp{os.getpid()}"
                with open(tmp, "wb") as f:
                    f.write(neff)
                os.replace(tmp, path)
            except Exception:
                pass
        return r

    bass2jax.neuronx_cc_hook = cached_hook
    bass2jax._ant_neff_cache_wrapped = True


_install_neff_cache()


def _build_key(C_r, C_s):
    import inspect

    src = inspect.getsource(_build) + inspect.getsource(_chunks)
    blob = f"moe-ep-v2|{C_r}|{C_s}|{sorted(CFG.items())}|{src}"
    return hashlib.sha256(blob.encode()).hexdigest()

BF16 = ml_dtypes.bfloat16
P = 128
H = 2048
F = 1408
E = 8
TOPK = 2
NCORES = 8
KH = H // P   # 16 contraction tiles over H
KF = F // P   # 11 contraction tiles over F
HT = H // P   # 16 output row tiles in phase B

FP32 = mybir.dt.float32
BF16_DT = mybir.dt.bfloat16


def _chunks(C, first=None):
    """Split C into 512-wide chunks (+ remainder).  N=512 matmuls amortize
    per-instruction overheads; `first` optionally shrinks the leading chunk
    so the kernel's first matmuls wait on a smaller x transfer."""
    out = []
    c0 = 0
    if first and first < C:
        out.append((0, first))
        c0 = first
    while c0 < C:
        cb = min(512, C - c0)
        out.append((c0, cb))
        c0 += cb
    return out


CFG = {
    "f_grp": 1,       # f-tiles per phase-A group (chunk loop inside)
    "w13_split": 2,   # dma_starts per w1f/w3f tile
    "w13_bufs": 4,
    "f0_wp": (4,),    # k-tiles per w13 piece in the interleaved f=0 load
    "f0_xp": (4,),    # k-tiles per x piece in the interleaved f=0 load
    "f0_order": (0, 1, 2),  # issue priority of (w1, x, w3) pieces per k-group
    "x0_eng": "sync",   # first-chunk x issue engine
    "x_grp": 4,       # k-tiles per x dma_start, non-first chunks
    "x_grp_pre": 8,   # k-tiles per x dma_start for prefetched problems
    "s_prefetch_x": False,  # batch-prefetch the shared problem's x early
    "w2_split": 1,    # dma_starts per w2 f-slice
    "w2_defer_f": 2,  # emit the w2 bulk load at this f iteration
    "out_split": 1,   # dma_starts per output tile
    "tail_split": 2,  # column pieces for the kernel's final output unit
    "tail_last": 0,   # if set: final unit splits [cb-tail_last, tail_last]
    "b_small_first": False,  # remainder chunk first in non-final phase B
    "tail_copy_alt": False,  # alternate tail-piece copies onto the Act engine
    "alt_copy_below": 0,  # alternate copies DVE/Act for units narrower than this
    "ps1_bufs": 3,
    "ps3_bufs": 3,
    "ps2_bufs": 2,
    "o_bufs": 10,
    "silu_bufs": 3,
    "dma_eng": "sync",  # w13 weight stream issue engine
    "x_eng": "sync",    # x load issue engine
    "w2_eng": "sync",   # bulk w2 load issue engine
    "out_engs": ("gpsimd", "scalar"),  # output store issue engines (rotated)
    "out_engs_s": ("sync",),  # store engines for the last problem (SP is idle
                              # during the final phase B; shorter DGE delay)
    "out_bf16": True,   # store outputs as bf16 (halves output DMA + tail)
    "shared_first": False,  # run the shared-expert problem first
    "chunk0": None,     # optional smaller leading chunk (startup latency)
    "warmup_mms": 0,    # dummy matmuls at t=0 (PE p-state warm-up)
}


def _split_dma(eng, dst, src, n):
    w = dst.shape[-1]
    step = -(-w // n)
    for i in range(0, w, step):
        j = min(w, i + step)
        eng.dma_start(dst[:, i:j], src[:, i:j])


def _build(nc, C_r, C_s):
    """Emit the per-core program: routed expert (C_r tokens, gate folded into
    xg) then the shared-expert slice (C_s tokens)."""
    dram = {}
    for name, shape, dt in [
        ("xr", [P, KH, C_r], BF16_DT),
        ("w1", [P, KF * KH * P], BF16_DT),
        ("w3", [P, KF * KH * P], BF16_DT),
        ("w2", [P, KF, H], BF16_DT),
        ("xs", [P, KH, C_s], BF16_DT),
        ("s1", [P, KF * KH * P], BF16_DT),
        ("s3", [P, KF * KH * P], BF16_DT),
        ("s2", [P, KF, H], BF16_DT),
    ]:
        dram[name] = nc.dram_tensor(name, shape, dt, kind="ExternalInput")
    out_dt = BF16_DT if CFG["out_bf16"] else FP32
    yr = nc.dram_tensor("yr", [H, C_r], out_dt, kind="ExternalOutput")
    ys = nc.dram_tensor("ys", [H, C_s], out_dt, kind="ExternalOutput")

    with tile.TileContext(nc) as tc, ExitStack() as ctx:
        pool = ctx.enter_context(tc.tile_pool(name="main", bufs=1))
        psum = ctx.enter_context(tc.tile_pool(name="ps", bufs=1, space="PSUM"))
        dmae = getattr(nc, CFG["dma_eng"])
        xeng = getattr(nc, CFG["x_eng"])
        w2eng = getattr(nc, CFG["w2_eng"])

        if CFG["warmup_mms"]:
            wz = pool.tile([P, P], BF16_DT, tag="warm_w", bufs=1)
            rz = pool.tile([P, 512], BF16_DT, tag="warm_r", bufs=1)
            nc.gpsimd.memset(wz[:], 0.0)
            nc.gpsimd.memset(rz[:], 0.0)
            pz = psum.tile([P, 512], FP32, tag="warm_ps", bufs=1)
            for _ in range(CFG["warmup_mms"]):
                nc.tensor.matmul(pz[:], lhsT=wz[:], rhs=rz[:], start=True,
                                 stop=True)

        def load_x(x_sb, xd, c0, cb, grp):
            """Load x columns [c0, c0+cb) for all KH k-tiles, `grp` k-tiles
            per dma_start (bigger groups cost one SEQ/HWDGE slot; smaller
            groups land sooner for the consuming matmuls)."""
            for k0 in range(0, KH, grp):
                k1 = min(KH, k0 + grp)
                xeng.dma_start(
                    x_sb[:, k0:k1, c0 : c0 + cb], xd[:, k0:k1, c0 : c0 + cb]
                )

        def problem(tag, xd, w1d, w3d, w2d, yd, C, first_chunk=None,
                    is_last=False, prefetch_x=False):
            # resident x
            x_sb = pool.tile([P, KH, C], BF16_DT, tag=f"x_{tag}", bufs=1)
            w2_sb = pool.tile([P, KF, H], BF16_DT, tag="w2", bufs=1)
            act_sb = pool.tile([P, KF, C], BF16_DT, tag=f"act_{tag}", bufs=1)
            if prefetch_x:
                # mid-kernel problem: x can stream during the previous
                # problem's phase B, so batch it instead of fine-piecing
                for c0, cb in _chunks(C, first=first_chunk):
                    load_x(x_sb, xd, c0, cb, CFG["x_grp_pre"])

            def emit_w13(f, w1f, w3f):
                if f == 0:
                    # startup-critical: interleave w1/x/w3 piece loads per
                    # k-group so the first matmuls start as soon as the first
                    # small pieces land and then stream.  Piece sizes (in
                    # k-tiles) can ramp up so the leading pieces land fast.
                    x0eng = getattr(nc, CFG["x0_eng"])
                    c0_, cb_ = chunks[0]

                    def pieces(sizes):
                        out, k = [], 0
                        for s in sizes:
                            if k >= KH:
                                break
                            s = min(s, KH - k)
                            out.append((k, k + s))
                            k += s
                        while k < KH:
                            out.append((k, min(KH, k + sizes[-1])))
                            k = min(KH, k + sizes[-1])
                        return out

                    ev = []
                    kw1, kx, kw3 = CFG["f0_order"]
                    for g0, g1 in pieces(CFG["f0_wp"]):
                        ev.append((g0, kw1, (g0, g1)))
                        ev.append((g0, kw3, (g0, g1)))
                    if not prefetch_x:
                        for g0, g1 in pieces(CFG["f0_xp"]):
                            ev.append((g0, kx, (g0, g1)))
                    # per k-coverage, in CFG["f0_order"] priority
                    for _, kind, (g0, g1) in sorted(ev, key=lambda t: (t[0], t[1])):
                        if kind == kw1:
                            dmae.dma_start(w1f[:, g0 * P : g1 * P],
                                           w1d[:, g0 * P : g1 * P])
                        elif kind == kw3:
                            dmae.dma_start(w3f[:, g0 * P : g1 * P],
                                           w3d[:, g0 * P : g1 * P])
                        else:
                            x0eng.dma_start(x_sb[:, g0:g1, c0_ : c0_ + cb_],
                                            xd[:, g0:g1, c0_ : c0_ + cb_])
                else:
                    wsplit = CFG["w13_split"]
                    _split_dma(dmae, w1f[:], w1d[:, f * KH * P : (f + 1) * KH * P], wsplit)
                    _split_dma(dmae, w3f[:], w3d[:, f * KH * P : (f + 1) * KH * P], wsplit)

            # ---- phase A: act[f, c] = silu(x@w1.T) * (x@w3.T), [F, C]
            # f-tiles are processed in groups of `f_grp` with the chunk loop
            # outside the within-group f loop: the first group then spans
            # several chunks of PE time, which is what lets the full x stream
            # (needed by every chunk at f=0) fit under the DMA bandwidth
            # bound.
            chunks = _chunks(C, first=first_chunk)
            FG = max(1, CFG["f_grp"])
            groups = [list(range(g, min(KF, g + FG))) for g in range(0, KF, FG)]
            for gi, fs in enumerate(groups):
                w1fs, w3fs = {}, {}
                for f in fs:
                    w1fs[f] = pool.tile([P, KH * P], BF16_DT, tag="w1f",
                                        bufs=CFG["w13_bufs"], name=f"w1f{f}")
                    w3fs[f] = pool.tile([P, KH * P], BF16_DT, tag="w3f",
                                        bufs=CFG["w13_bufs"], name=f"w3f{f}")
                    emit_w13(f, w1fs[f], w3fs[f])
                    if f == CFG["w2_defer_f"]:
                        # defer the (large, phase-B-only) w2 load past startup
                        for ff in range(KF):
                            _split_dma(w2eng, w2_sb[:, ff, :], w2d[:, ff, :],
                                       CFG["w2_split"])
                for ci, (c0, cb) in enumerate(chunks):
                    if gi == 0 and ci > 0 and not prefetch_x:
                        load_x(x_sb, xd, c0, cb, CFG["x_grp"])
                    for f in fs:
                        w1f, w3f = w1fs[f], w3fs[f]
                        ps1 = psum.tile([P, 512], FP32, tag="ps1", bufs=CFG["ps1_bufs"])
                        ps3 = psum.tile([P, 512], FP32, tag="ps3", bufs=CFG["ps3_bufs"])
                        # interleaved per k: each x piece feeds w1 then w3
                        for kk in range(KH):
                            nc.tensor.matmul(
                                ps1[:, :cb],
                                lhsT=w1f[:, kk * P : (kk + 1) * P],
                                rhs=x_sb[:, kk, c0 : c0 + cb],
                                start=(kk == 0),
                                stop=(kk == KH - 1),
                            )
                            nc.tensor.matmul(
                                ps3[:, :cb],
                                lhsT=w3f[:, kk * P : (kk + 1) * P],
                                rhs=x_sb[:, kk, c0 : c0 + cb],
                                start=(kk == 0),
                                stop=(kk == KH - 1),
                            )
                        tmp = pool.tile([P, 512], BF16_DT, tag="silu",
                                        bufs=CFG["silu_bufs"])
                        nc.scalar.activation(
                            tmp[:, :cb], ps1[:, :cb],
                            mybir.ActivationFunctionType.Silu
                        )
                        nc.vector.tensor_mul(
                            act_sb[:, f, c0 : c0 + cb], tmp[:, :cb], ps3[:, :cb]
                        )

            # ---- phase B: y[h, c] = w2[f,h].T @ act[f,c], h-major output
            # phase A's psum rings are idle here, so alternate between them
            # for extra store pipelining depth
            oengs = [getattr(nc, e)
                     for e in (CFG["out_engs_s"] if is_last else CFG["out_engs"])]
            bchunks = _chunks(C)
            if CFG["b_small_first"] and not is_last and len(bchunks) > 1:
                # fire the remainder chunk's tiny stores early, away from the
                # next problem's silu traffic on the store queues
                bchunks.sort(key=lambda c: c[1])
            i = 0
            for ci, (c0, cb) in enumerate(bchunks):
                for ht in range(HT):
                    # the very last unit of the last problem is the kernel's
                    # tail: split it into column pieces so the final
                    # copy+store chain is short
                    tail = (is_last and ht == HT - 1
                            and ci == len(bchunks) - 1 and cb > 64)
                    if tail and CFG["tail_last"]:
                        cuts = [0, cb - CFG["tail_last"], cb]
                    elif tail:
                        n = CFG["tail_split"]
                        cuts = [min(cb, i * -(-cb // n)) for i in range(n + 1)]
                    else:
                        cuts = [0, cb]
                    for p0, p1_ in zip(cuts, cuts[1:]):
                        if p1_ <= p0:
                            continue
                        pb = p1_ - p0
                        ps2 = psum.tile([P, 512], FP32, tag="ps2",
                                        bufs=CFG["ps2_bufs"])
                        for f in range(KF):
                            nc.tensor.matmul(
                                ps2[:, :pb],
                                lhsT=w2_sb[:, f, ht * P : (ht + 1) * P],
                                rhs=act_sb[:, f, c0 + p0 : c0 + p0 + pb],
                                start=(f == 0),
                                stop=(f == KF - 1),
                            )
                        o = pool.tile([P, 512], out_dt, tag="o",
                                      bufs=CFG["o_bufs"])
                        if (tail and p0 > 0 and CFG["tail_copy_alt"]) or (
                                cb <= CFG["alt_copy_below"] and i % 2 == 1):
                            # small units: alternate copies onto the Act
                            # engine so they don't serialize on DVE
                            nc.scalar.activation(
                                o[:, :pb], ps2[:, :pb],
                                mybir.ActivationFunctionType.Copy)
                        else:
                            nc.vector.tensor_copy(o[:, :pb], ps2[:, :pb])
                        _split_dma(
                            oengs[i % len(oengs)],
                            yd[ht * P : (ht + 1) * P, c0 + p0 : c0 + p0 + pb],
                            o[:, :pb],
                            CFG["out_split"],
                        )
                        i += 1

        rargs = ("r", dram["xr"].ap(), dram["w1"].ap(),
                 dram["w3"].ap(), dram["w2"].ap(), yr.ap(), C_r)
        sargs = ("s", dram["xs"].ap(), dram["s1"].ap(),
                 dram["s3"].ap(), dram["s2"].ap(), ys.ap(), C_s)
        if CFG["shared_first"]:
            problem(*sargs)
            problem(*rargs, first_chunk=CFG["chunk0"], is_last=True)
        else:
            problem(*rargs, first_chunk=CFG["chunk0"])
            problem(*sargs, is_last=True, prefetch_x=CFG["s_prefetch_x"])

    return nc


_cache = {}


def _get_nc(C_r, C_s):
    key = (C_r, C_s, tuple(sorted(CFG.items())))
    if key not in _cache:
        nc = bacc.Bacc("TRN2", target_bir_lowering=False, debug=False,
                       num_devices=NCORES)
        _build(nc, C_r, C_s)
        nc.compile()
        _cache[key] = nc
    return _cache[key]


def _tile_w13(w):
    """[F, H] fp32 -> [128, KF*KH*128] bf16, (f, kk, j) column order."""
    a = np.ascontiguousarray(w, np.float32).astype(BF16)
    return np.ascontiguousarray(
        a.reshape(KF, P, KH, P).transpose(3, 0, 2, 1)
    ).reshape(P, KF * KH * P)


def _tile_w2(w):
    """[H, F] fp32 -> [128, KF, H] bf16, (f, h) column order."""
    a = np.ascontiguousarray(w, np.float32).astype(BF16)
    return np.ascontiguousarray(a.reshape(H, KF, P).transpose(2, 1, 0))


def _pad_rows(x, n):
    if x.shape[0] == n:
        return x
    out = np.zeros((n, x.shape[1]), x.dtype)
    out[: x.shape[0]] = x
    return out


def _tile_x(x):
    """[C, H] fp32 -> [128, KH, C] bf16, (kk, c) column order."""
    C = x.shape[0]
    a = x.astype(BF16)
    return np.ascontiguousarray(a.reshape(C, KH, P).transpose(2, 1, 0))


def kernel(hidden_states, gate_w, bias, ws1, ws2, ws3, we1, we2, we3):
    orig_shape = hidden_states.shape
    x = np.ascontiguousarray(
        np.asarray(hidden_states, np.float32).reshape(-1, orig_shape[-1])
    )
    T = x.shape[0]
    gate_w = np.asarray(gate_w, np.float32)
    bias = np.asarray(bias, np.float32)
    we1 = np.asarray(we1, np.float32)
    we2 = np.asarray(we2, np.float32)
    we3 = np.asarray(we3, np.float32)
    assert gate_w.shape[0] == E and we1.shape[0] == E and x.shape[1] == H

    # ---- host router (fp32, matches the reference's selection math)
    logits = x @ gate_w.T                                 # [T, E]
    scores = np.where(
        logits >= 0,
        1.0 / (1.0 + np.exp(-np.abs(logits))),
        1.0 - 1.0 / (1.0 + np.exp(-np.abs(logits))),
    ).astype(np.float32)
    routing = scores + bias[None, :]
    topk = np.argsort(-routing, axis=1, kind="stable")[:, :TOPK]  # [T, K]
    sel = np.take_along_axis(scores, topk, axis=1)
    gates = sel / sel.sum(axis=1, keepdims=True)          # [T, K]

    idx_e = []      # token ids routed to expert e
    gate_e = []     # matching combine weights
    for e in range(E):
        mask = topk == e                      # [T, K], at most one True per row
        rows = np.nonzero(mask.any(axis=1))[0]
        idx_e.append(rows)
        gate_e.append(gates[mask].astype(np.float32))  # row-major -> rows order

    C_r = max(1, max(len(r) for r in idx_e))   # exact routed capacity
    C_s = -(-T // NCORES)                      # shared tokens per core

    nc = _get_nc(C_r, C_s)

    # ---- build per-core input maps
    shared_w = {
        "s1": _tile_w13(ws1),
        "s3": _tile_w13(ws3),
        "s2": _tile_w2(ws2),
    }
    in_maps = []
    for e in range(E):
        rows = idx_e[e]
        xfull = np.zeros((C_r, H), np.float32)
        xfull[: len(rows)] = x[rows]
        m = {
            "xr": _tile_x(xfull),
            "w1": _tile_w13(we1[e]),
            "w3": _tile_w13(we3[e]),
            "w2": _tile_w2(we2[e]),
            "xs": _tile_x(_pad_rows(x[e * C_s : (e + 1) * C_s], C_s)),
        }
        m.update(shared_w)
        in_maps.append(m)

    global _active_build_key
    _active_build_key = _build_key(C_r, C_s)
    try:
        res = run_bass_kernel_spmd(nc, in_maps, list(range(NCORES))).results
    finally:
        _active_build_key = None

    # ---- host combine (outputs are [H, C] h-major; gate applied here)
    out = np.zeros((T, H), np.float32)
    for e in range(E):
        rows = idx_e[e]
        yr = np.asarray(res[e]["yr"][:, : len(rows)], np.float32)
        out[rows] += (yr * gate_e[e][None, :]).T
        lo = e * C_s
        hi = min(T, (e + 1) * C_s)
        if lo < hi:
            out[lo:hi] += np.asarray(res[e]["ys"][:, : hi - lo], np.float32).T
    return out.reshape(orig_shape).astype(np.float32)


# revision 56
# speedup vs baseline: 1.0016x; 1.0016x over previous
"""DeepSeekV3-style MoE block on 8 Trainium2 NeuronCores.

Strategy (expert-parallel, host-routed dispatch/combine):
  - Host computes the (tiny) sigmoid gate in fp32 numpy, does top-2 selection
    and builds per-expert token lists (the "all-to-all dispatch" happens while
    sharding the inputs).
  - Core e runs expert e's SwiGLU over its gathered tokens (padded to the
    exact max expert load) plus a 1/8 token-slice of the shared expert, all
    in bf16 on the TensorEngine with fp32 PSUM accumulation.
  - The per-token gate weight is applied on the HOST during the combine
    scatter-add (it is a per-column scale of the core's output), so the
    device program is a pure unscaled SwiGLU.
  - The host scatter-adds the per-core outputs back into the full [B,S,H]
    tensor (the "combine" happens while unsharding).

Matmul layouts (no on-chip transposes):
  phase A:  act[f,c] = silu(w1[h,f].T @ x[h,c]) * (w3[h,f].T @ x[h,c])
            (w1/w3 matmuls interleaved per k-tile so each freshly-landed x
            piece feeds two back-to-back matmuls — matches the DMA issue
            rate at f=0 when x is still streaming in)
  phase B:  y[h,c]   = w2[f,h].T @ act[f,c]     (h-major output: cost tracks
            the exact token count instead of 128-rounded token tiles)
Host-side pre-tiling puts every DRAM operand in [128, ...] partition-major
layout so each DMA is contiguous.
"""

import hashlib
import os
import sys

for _p in ("/opt/trn_rl_repo", "/opt/pypackages"):
    if _p not in sys.path:
        sys.path.append(_p)

from contextlib import ExitStack

import numpy as np
import ml_dtypes

import concourse.bacc as bacc
import concourse.mybir as mybir
import concourse.tile as tile
from concourse import bass2jax
from concourse.bass_utils import run_bass_kernel_spmd

_NEFF_CACHE_DIR = os.path.expanduser("~/.cache/bass_neff_cache")
_active_build_key = None   # set by _get_nc around the PJRT dispatch


def _install_neff_cache():
    """Persist the compiled bass_exec NEFF across processes.

    The walrus backend takes minutes for this kernel and has no cache of its
    own.  The HLO bytes are not byte-stable across processes (volatile ids /
    debug metadata), so the cache key is derived from the *build inputs*
    (capacities + CFG + build source) instead.  Only the renamed NEFF bytes
    are stored; each request re-wraps them around its own HLO."""
    if getattr(bass2jax, "_ant_neff_cache_wrapped", False):
        return
    inner = bass2jax.neuronx_cc_hook

    captured = {}
    orig_rename = bass2jax.rename_neff_tensors_and_patch_header

    def capture_rename(neff_path, mapping):
        data = orig_rename(neff_path, mapping)
        captured["neff"] = data
        return data

    bass2jax.rename_neff_tensors_and_patch_header = capture_rename

    def cached_hook(code, code_format, platform_version, file_prefix):
        c = code if isinstance(code, (bytes, bytearray)) else str(code).encode()
        if b"bass_exec" not in c or _active_build_key is None:
            return inner(code, code_format, platform_version, file_prefix)
        from libneuronxla.libncc import _wrap_neff_as_custom_call

        path = os.path.join(_NEFF_CACHE_DIR, _active_build_key + ".neff")
        try:
            if os.path.exists(path):
                with open(path, "rb") as f:
                    return 0, _wrap_neff_as_custom_call(bytes(c), f.read())
        except Exception:
            pass
        captured.pop("neff", None)
        r = inner(code, code_format, platform_version, file_prefix)
        neff = captured.pop("neff", None)
        if neff is not None:
            try:
                os.makedirs(_NEFF_CACHE_DIR, exist_ok=True)
                tmp = f"{path}.tmp{os.getpid()}"
                with open(tmp, "wb") as f:
                    f.write(neff)
                os.replace(tmp, path)
            except Exception:
                pass
        return r

    bass2jax.neuronx_cc_hook = cached_hook
    bass2jax._ant_neff_cache_wrapped = True


_install_neff_cache()


def _build_key(C_r, C_s):
    import inspect

    src = inspect.getsource(_build) + inspect.getsource(_chunks)
    blob = f"moe-ep-v2|{C_r}|{C_s}|{sorted(CFG.items())}|{src}"
    return hashlib.sha256(blob.encode()).hexdigest()

BF16 = ml_dtypes.bfloat16
P = 128
H = 2048
F = 1408
E = 8
TOPK = 2
NCORES = 8
KH = H // P   # 16 contraction tiles over H
KF = F // P   # 11 contraction tiles over F
HT = H // P   # 16 output row tiles in phase B

FP32 = mybir.dt.float32
BF16_DT = mybir.dt.bfloat16


def _chunks(C, first=None):
    """Split C into 512-wide chunks (+ remainder).  N=512 matmuls amortize
    per-instruction overheads; `first` optionally shrinks the leading chunk
    so the kernel's first matmuls wait on a smaller x transfer."""
    out = []
    c0 = 0
    if first and first < C:
        out.append((0, first))
        c0 = first
    while c0 < C:
        cb = min(512, C - c0)
        out.append((c0, cb))
        c0 += cb
    return out


CFG = {
    "f_grp": 1,       # f-tiles per phase-A group (chunk loop inside)
    "w13_split": 2,   # dma_starts per w1f/w3f tile
    "w13_bufs": 4,
    "f0_wp": (4,),    # k-tiles per w13 piece in the interleaved f=0 load
    "f0_xp": (4,),    # k-tiles per x piece in the interleaved f=0 load
    "f0_order": (0, 1, 2),  # issue priority of (w1, x, w3) pieces per k-group
    "x0_eng": "sync",   # first-chunk x issue engine
    "x_grp": 4,       # k-tiles per x dma_start, non-first chunks
    "x_grp_pre": 8,   # k-tiles per x dma_start for prefetched problems
    "s_prefetch_x": False,  # batch-prefetch the shared problem's x early
    "w2_split": 1,    # dma_starts per w2 f-slice
    "w2_defer_f": 2,  # emit the w2 bulk load at this f iteration
    "out_split": 1,   # dma_starts per output tile
    "tail_split": 2,  # column pieces for the kernel's final output unit
    "tail_last": 0,   # if set: final unit splits [cb-tail_last, tail_last]
    "b_small_first": False,  # remainder chunk first in non-final phase B
    "tail_copy_alt": False,  # alternate tail-piece copies onto the Act engine
    "alt_copy_below": 0,  # alternate copies DVE/Act for units narrower than this
    "ps_borrow_below": 64,  # tiny phase-B units borrow phase-A psum rings
    "ps1_bufs": 3,
    "ps3_bufs": 3,
    "ps2_bufs": 2,
    "o_bufs": 10,
    "silu_bufs": 3,
    "dma_eng": "sync",  # w13 weight stream issue engine
    "x_eng": "sync",    # x load issue engine
    "w2_eng": "sync",   # bulk w2 load issue engine
    "out_engs": ("gpsimd", "scalar"),  # output store issue engines (rotated)
    "out_engs_s": ("sync",),  # store engines for the last problem (SP is idle
                              # during the final phase B; shorter DGE delay)
    "out_bf16": True,   # store outputs as bf16 (halves output DMA + tail)
    "shared_first": False,  # run the shared-expert problem first
    "chunk0": None,     # optional smaller leading chunk (startup latency)
    "warmup_mms": 0,    # dummy matmuls at t=0 (PE p-state warm-up)
}


def _split_dma(eng, dst, src, n):
    w = dst.shape[-1]
    step = -(-w // n)
    for i in range(0, w, step):
        j = min(w, i + step)
        eng.dma_start(dst[:, i:j], src[:, i:j])


def _build(nc, C_r, C_s):
    """Emit the per-core program: routed expert (C_r tokens, gate folded into
    xg) then the shared-expert slice (C_s tokens)."""
    dram = {}
    for name, shape, dt in [
        ("xr", [P, KH, C_r], BF16_DT),
        ("w1", [P, KF * KH * P], BF16_DT),
        ("w3", [P, KF * KH * P], BF16_DT),
        ("w2", [P, KF, H], BF16_DT),
        ("xs", [P, KH, C_s], BF16_DT),
        ("s1", [P, KF * KH * P], BF16_DT),
        ("s3", [P, KF * KH * P], BF16_DT),
        ("s2", [P, KF, H], BF16_DT),
    ]:
        dram[name] = nc.dram_tensor(name, shape, dt, kind="ExternalInput")
    out_dt = BF16_DT if CFG["out_bf16"] else FP32
    yr = nc.dram_tensor("yr", [H, C_r], out_dt, kind="ExternalOutput")
    ys = nc.dram_tensor("ys", [H, C_s], out_dt, kind="ExternalOutput")

    with tile.TileContext(nc) as tc, ExitStack() as ctx:
        pool = ctx.enter_context(tc.tile_pool(name="main", bufs=1))
        psum = ctx.enter_context(tc.tile_pool(name="ps", bufs=1, space="PSUM"))
        dmae = getattr(nc, CFG["dma_eng"])
        xeng = getattr(nc, CFG["x_eng"])
        w2eng = getattr(nc, CFG["w2_eng"])

        if CFG["warmup_mms"]:
            wz = pool.tile([P, P], BF16_DT, tag="warm_w", bufs=1)
            rz = pool.tile([P, 512], BF16_DT, tag="warm_r", bufs=1)
            nc.gpsimd.memset(wz[:], 0.0)
            nc.gpsimd.memset(rz[:], 0.0)
            pz = psum.tile([P, 512], FP32, tag="warm_ps", bufs=1)
            for _ in range(CFG["warmup_mms"]):
                nc.tensor.matmul(pz[:], lhsT=wz[:], rhs=rz[:], start=True,
                                 stop=True)

        def load_x(x_sb, xd, c0, cb, grp):
            """Load x columns [c0, c0+cb) for all KH k-tiles, `grp` k-tiles
            per dma_start (bigger groups cost one SEQ/HWDGE slot; smaller
            groups land sooner for the consuming matmuls)."""
            for k0 in range(0, KH, grp):
                k1 = min(KH, k0 + grp)
                xeng.dma_start(
                    x_sb[:, k0:k1, c0 : c0 + cb], xd[:, k0:k1, c0 : c0 + cb]
                )

        def problem(tag, xd, w1d, w3d, w2d, yd, C, first_chunk=None,
                    is_last=False, prefetch_x=False):
            # resident x
            x_sb = pool.tile([P, KH, C], BF16_DT, tag=f"x_{tag}", bufs=1)
            w2_sb = pool.tile([P, KF, H], BF16_DT, tag="w2", bufs=1)
            act_sb = pool.tile([P, KF, C], BF16_DT, tag=f"act_{tag}", bufs=1)
            if prefetch_x:
                # mid-kernel problem: x can stream during the previous
                # problem's phase B, so batch it instead of fine-piecing
                for c0, cb in _chunks(C, first=first_chunk):
                    load_x(x_sb, xd, c0, cb, CFG["x_grp_pre"])

            def emit_w13(f, w1f, w3f):
                if f == 0:
                    # startup-critical: interleave w1/x/w3 piece loads per
                    # k-group so the first matmuls start as soon as the first
                    # small pieces land and then stream.  Piece sizes (in
                    # k-tiles) can ramp up so the leading pieces land fast.
                    x0eng = getattr(nc, CFG["x0_eng"])
                    c0_, cb_ = chunks[0]

                    def pieces(sizes):
                        out, k = [], 0
                        for s in sizes:
                            if k >= KH:
                                break
                            s = min(s, KH - k)
                            out.append((k, k + s))
                            k += s
                        while k < KH:
                            out.append((k, min(KH, k + sizes[-1])))
                            k = min(KH, k + sizes[-1])
                        return out

                    ev = []
                    kw1, kx, kw3 = CFG["f0_order"]
                    for g0, g1 in pieces(CFG["f0_wp"]):
                        ev.append((g0, kw1, (g0, g1)))
                        ev.append((g0, kw3, (g0, g1)))
                    if not prefetch_x:
                        for g0, g1 in pieces(CFG["f0_xp"]):
                            ev.append((g0, kx, (g0, g1)))
                    # per k-coverage, in CFG["f0_order"] priority
                    for _, kind, (g0, g1) in sorted(ev, key=lambda t: (t[0], t[1])):
                        if kind == kw1:
                            dmae.dma_start(w1f[:, g0 * P : g1 * P],
                                           w1d[:, g0 * P : g1 * P])
                        elif kind == kw3:
                            dmae.dma_start(w3f[:, g0 * P : g1 * P],
                                           w3d[:, g0 * P : g1 * P])
                        else:
                            x0eng.dma_start(x_sb[:, g0:g1, c0_ : c0_ + cb_],
                                            xd[:, g0:g1, c0_ : c0_ + cb_])
                else:
                    wsplit = CFG["w13_split"]
                    _split_dma(dmae, w1f[:], w1d[:, f * KH * P : (f + 1) * KH * P], wsplit)
                    _split_dma(dmae, w3f[:], w3d[:, f * KH * P : (f + 1) * KH * P], wsplit)

            # ---- phase A: act[f, c] = silu(x@w1.T) * (x@w3.T), [F, C]
            # f-tiles are processed in groups of `f_grp` with the chunk loop
            # outside the within-group f loop: the first group then spans
            # several chunks of PE time, which is what lets the full x stream
            # (needed by every chunk at f=0) fit under the DMA bandwidth
            # bound.
            chunks = _chunks(C, first=first_chunk)
            FG = max(1, CFG["f_grp"])
            groups = [list(range(g, min(KF, g + FG))) for g in range(0, KF, FG)]
            for gi, fs in enumerate(groups):
                w1fs, w3fs = {}, {}
                for f in fs:
                    w1fs[f] = pool.tile([P, KH * P], BF16_DT, tag="w1f",
                                        bufs=CFG["w13_bufs"], name=f"w1f{f}")
                    w3fs[f] = pool.tile([P, KH * P], BF16_DT, tag="w3f",
                                        bufs=CFG["w13_bufs"], name=f"w3f{f}")
                    emit_w13(f, w1fs[f], w3fs[f])
                    if f == CFG["w2_defer_f"]:
                        # defer the (large, phase-B-only) w2 load past startup
                        for ff in range(KF):
                            _split_dma(w2eng, w2_sb[:, ff, :], w2d[:, ff, :],
                                       CFG["w2_split"])
                for ci, (c0, cb) in enumerate(chunks):
                    if gi == 0 and ci > 0 and not prefetch_x:
                        load_x(x_sb, xd, c0, cb, CFG["x_grp"])
                    for f in fs:
                        w1f, w3f = w1fs[f], w3fs[f]
                        ps1 = psum.tile([P, 512], FP32, tag="ps1", bufs=CFG["ps1_bufs"])
                        ps3 = psum.tile([P, 512], FP32, tag="ps3", bufs=CFG["ps3_bufs"])
                        # interleaved per k: each x piece feeds w1 then w3
                        for kk in range(KH):
                            nc.tensor.matmul(
                                ps1[:, :cb],
                                lhsT=w1f[:, kk * P : (kk + 1) * P],
                                rhs=x_sb[:, kk, c0 : c0 + cb],
                                start=(kk == 0),
                                stop=(kk == KH - 1),
                            )
                            nc.tensor.matmul(
                                ps3[:, :cb],
                                lhsT=w3f[:, kk * P : (kk + 1) * P],
                                rhs=x_sb[:, kk, c0 : c0 + cb],
                                start=(kk == 0),
                                stop=(kk == KH - 1),
                            )
                        tmp = pool.tile([P, 512], BF16_DT, tag="silu",
                                        bufs=CFG["silu_bufs"])
                        nc.scalar.activation(
                            tmp[:, :cb], ps1[:, :cb],
                            mybir.ActivationFunctionType.Silu
                        )
                        nc.vector.tensor_mul(
                            act_sb[:, f, c0 : c0 + cb], tmp[:, :cb], ps3[:, :cb]
                        )

            # ---- phase B: y[h, c] = w2[f,h].T @ act[f,c], h-major output
            # phase A's psum rings are idle here, so alternate between them
            # for extra store pipelining depth
            oengs = [getattr(nc, e)
                     for e in (CFG["out_engs_s"] if is_last else CFG["out_engs"])]
            bchunks = _chunks(C)
            if CFG["b_small_first"] and not is_last and len(bchunks) > 1:
                # fire the remainder chunk's tiny stores early, away from the
                # next problem's silu traffic on the store queues
                bchunks.sort(key=lambda c: c[1])
            i = 0
            for ci, (c0, cb) in enumerate(bchunks):
                for ht in range(HT):
                    # the very last unit of the last problem is the kernel's
                    # tail: split it into column pieces so the final
                    # copy+store chain is short
                    tail = (is_last and ht == HT - 1
                            and ci == len(bchunks) - 1 and cb > 64)
                    if tail and CFG["tail_last"]:
                        cuts = [0, cb - CFG["tail_last"], cb]
                    elif tail:
                        n = CFG["tail_split"]
                        cuts = [min(cb, i * -(-cb // n)) for i in range(n + 1)]
                    else:
                        cuts = [0, cb]
                    for p0, p1_ in zip(cuts, cuts[1:]):
                        if p1_ <= p0:
                            continue
                        pb = p1_ - p0
                        if cb <= CFG["ps_borrow_below"] and not is_last:
                            # tiny remainder units: borrow the (idle) phase-A
                            # psum rings for extra copy-pipelining depth
                            bt = ("ps1", "ps3", "ps2")[i % 3]
                            bb = {"ps1": CFG["ps1_bufs"], "ps3": CFG["ps3_bufs"],
                                  "ps2": CFG["ps2_bufs"]}[bt]
                            ps2 = psum.tile([P, 512], FP32, tag=bt, bufs=bb,
                                            name="ps2b")
                        else:
                            ps2 = psum.tile([P, 512], FP32, tag="ps2",
                                            bufs=CFG["ps2_bufs"])
                        for f in range(KF):
                            nc.tensor.matmul(
                                ps2[:, :pb],
                                lhsT=w2_sb[:, f, ht * P : (ht + 1) * P],
                                rhs=act_sb[:, f, c0 + p0 : c0 + p0 + pb],
                                start=(f == 0),
                                stop=(f == KF - 1),
                            )
                        o = pool.tile([P, 512], out_dt, tag="o",
                                      bufs=CFG["o_bufs"])
                        if (tail and p0 > 0 and CFG["tail_copy_alt"]) or (
                                cb <= CFG["alt_copy_below"] and i % 2 == 1):
                            # small units: alternate copies onto the Act
                            # engine so they don't serialize on DVE
                            nc.scalar.activation(
                                o[:, :pb], ps2[:, :pb],
                                mybir.ActivationFunctionType.Copy)
                        else:
                            nc.vector.tensor_copy(o[:, :pb], ps2[:, :pb])
                        _split_dma(
                            oengs[i % len(oengs)],
                            yd[ht * P : (ht + 1) * P, c0 + p0 : c0 + p0 + pb],
                            o[:, :pb],
                            CFG["out_split"],
                        )
                        i += 1

        rargs = ("r", dram["xr"].ap(), dram["w1"].ap(),
                 dram["w3"].ap(), dram["w2"].ap(), yr.ap(), C_r)
        sargs = ("s", dram["xs"].ap(), dram["s1"].ap(),
                 dram["s3"].ap(), dram["s2"].ap(), ys.ap(), C_s)
        if CFG["shared_first"]:
            problem(*sargs)
            problem(*rargs, first_chunk=CFG["chunk0"], is_last=True)
        else:
            problem(*rargs, first_chunk=CFG["chunk0"])
            problem(*sargs, is_last=True, prefetch_x=CFG["s_prefetch_x"])

    return nc


_cache = {}


def _get_nc(C_r, C_s):
    key = (C_r, C_s, tuple(sorted(CFG.items())))
    if key not in _cache:
        nc = bacc.Bacc("TRN2", target_bir_lowering=False, debug=False,
                       num_devices=NCORES)
        _build(nc, C_r, C_s)
        nc.compile()
        _cache[key] = nc
    return _cache[key]


def _tile_w13(w):
    """[F, H] fp32 -> [128, KF*KH*128] bf16, (f, kk, j) column order."""
    a = np.ascontiguousarray(w, np.float32).astype(BF16)
    return np.ascontiguousarray(
        a.reshape(KF, P, KH, P).transpose(3, 0, 2, 1)
    ).reshape(P, KF * KH * P)


def _tile_w2(w):
    """[H, F] fp32 -> [128, KF, H] bf16, (f, h) column order."""
    a = np.ascontiguousarray(w, np.float32).astype(BF16)
    return np.ascontiguousarray(a.reshape(H, KF, P).transpose(2, 1, 0))


def _pad_rows(x, n):
    if x.shape[0] == n:
        return x
    out = np.zeros((n, x.shape[1]), x.dtype)
    out[: x.shape[0]] = x
    return out


def _tile_x(x):
    """[C, H] fp32 -> [128, KH, C] bf16, (kk, c) column order."""
    C = x.shape[0]
    a = x.astype(BF16)
    return np.ascontiguousarray(a.reshape(C, KH, P).transpose(2, 1, 0))


def kernel(hidden_states, gate_w, bias, ws1, ws2, ws3, we1, we2, we3):
    orig_shape = hidden_states.shape
    x = np.ascontiguousarray(
        np.asarray(hidden_states, np.float32).reshape(-1, orig_shape[-1])
    )
    T = x.shape[0]
    gate_w = np.asarray(gate_w, np.float32)
    bias = np.asarray(bias, np.float32)
    we1 = np.asarray(we1, np.float32)
    we2 = np.asarray(we2, np.float32)
    we3 = np.asarray(we3, np.float32)
    assert gate_w.shape[0] == E and we1.shape[0] == E and x.shape[1] == H

    # ---- host router (fp32, matches the reference's selection math)
    logits = x @ gate_w.T                                 # [T, E]
    scores = np.where(
        logits >= 0,
        1.0 / (1.0 + np.exp(-np.abs(logits))),
        1.0 - 1.0 / (1.0 + np.exp(-np.abs(logits))),
    ).astype(np.float32)
    routing = scores + bias[None, :]
    topk = np.argsort(-routing, axis=1, kind="stable")[:, :TOPK]  # [T, K]
    sel = np.take_along_axis(scores, topk, axis=1)
    gates = sel / sel.sum(axis=1, keepdims=True)          # [T, K]

    idx_e = []      # token ids routed to expert e
    gate_e = []     # matching combine weights
    for e in range(E):
        mask = topk == e                      # [T, K], at most one True per row
        rows = np.nonzero(mask.any(axis=1))[0]
        idx_e.append(rows)
        gate_e.append(gates[mask].astype(np.float32))  # row-major -> rows order

    C_r = max(1, max(len(r) for r in idx_e))   # exact routed capacity
    C_s = -(-T // NCORES)                      # shared tokens per core

    nc = _get_nc(C_r, C_s)

    # ---- build per-core input maps
    shared_w = {
        "s1": _tile_w13(ws1),
        "s3": _tile_w13(ws3),
        "s2": _tile_w2(ws2),
    }
    in_maps = []
    for e in range(E):
        rows = idx_e[e]
        xfull = np.zeros((C_r, H), np.float32)
        xfull[: len(rows)] = x[rows]
        m = {
            "xr": _tile_x(xfull),
            "w1": _tile_w13(we1[e]),
            "w3": _tile_w13(we3[e]),
            "w2": _tile_w2(we2[e]),
            "xs": _tile_x(_pad_rows(x[e * C_s : (e + 1) * C_s], C_s)),
        }
        m.update(shared_w)
        in_maps.append(m)

    global _active_build_key
    _active_build_key = _build_key(C_r, C_s)
    try:
        res = run_bass_kernel_spmd(nc, in_maps, list(range(NCORES))).results
    finally:
        _active_build_key = None

    # ---- host combine (outputs are [H, C] h-major; gate applied here)
    out = np.zeros((T, H), np.float32)
    for e in range(E):
        rows = idx_e[e]
        yr = np.asarray(res[e]["yr"][:, : len(rows)], np.float32)
        out[rows] += (yr * gate_e[e][None, :]).T
        lo = e * C_s
        hi = min(T, (e + 1) * C_s)
        if lo < hi:
            out[lo:hi] += np.asarray(res[e]["ys"][:, : hi - lo], np.float32).T
    return out.reshape(orig_shape).astype(np.float32)
